# revision 12
# baseline (speedup 1.0000x reference)
"""Bidirectional Time-aware LSTM (TLSTM) for Trainium2, 8 NeuronCores.

Strategy: sequence-chunked parallelism, sharded by SEQUENCE position so the
two directions share one x upload. Core j owns positions [256j, 256j+256):
its F-stream runs forward windows (2j, 2j+1), its R-stream runs reverse
windows (15-2j, 14-2j) - both read the same on-device x pool of 304
positions ([256j-24, 256j+280), zero-padded outside [0, 2048)).

Windows are recomputed exactly by starting WARM=24 steps early from zero
state (the forget gate contracts state error ~0.6/step, so 24 steps reach
well below the bf16 noise floor). All 16 windows per direction use the
uniform start t0(w) = 128w - WARM; window 0 warms up over zero-padded
positions and converges onto the true trajectory, and the first WARM kept
positions of each direction are recomputed exactly on the host (they are
the only outputs that genuinely depend on h0/c0).

Host->device traffic dominates the cost, so the kernel ships raw x once
(bf16, feature-major pool blocks of [128 x 64]) and computes x @ W_ih^T
on-device each step with the step's x as the PE stationary operand; each
stream's paired-window stationary tiles are assembled from the shared pool
by strided on-device DMA. Biases ship as [1, n] rows and are partition-
broadcast on device. Outputs ship back as mu-law companded int8
(q = round(127 * ln(1+MU*|h|)/ln(1+MU)) * sign(h)), kept steps only;
the host decodes exactly.

Per stream per step (batch-major rows = 2x64 batch):
  psum_m[:, :128] = b_d (bcast const) + c @ W_d^T          (bf16 path)
  psum_g[:, :512] = b (bcast const) + xT_t stationary @ W_ih^T + h @ W_hh^T
  c_s = tanh(psum_m); sig_if/sig_o = sigmoid(gates); tg = tanh(g-block)
  c' = sig_f*(c + c_s*(r-1)) + sig_i*tg ;  h' = sig_o*tanh(c')
  c'/h' transposed on PE back to feature-major (bf16) for the next step.
"""

import os
import sys

import numpy as np
import ml_dtypes

for _p in ("/opt/trn_rl_repo",):
    if _p not in sys.path and os.path.isdir(_p):
        sys.path.insert(0, _p)

BF16 = ml_dtypes.bfloat16

S, B, I, H = 2048, 64, 128, 128
E = float(np.e)
NCORES = 8
N_STR = 2                     # streams per core: 0 = forward, 1 = reverse
N_WIN = 16                    # windows per direction
L_KEEP = S // N_WIN           # 128 kept steps per window
WARM = int(os.environ.get('TL_WARM', '24'))
L = L_KEEP + WARM             # wall steps per stream
POOL = 2 * L_KEEP + 2 * WARM  # x pool positions per core (304)
CH = 8                        # x-chunk steps per input DMA
STG = 8                       # output staging steps per output DMA

OUT_I8 = os.environ.get('TL_OUT', 'i8') == 'i8'
MU = 10.0
LQ = float(np.log1p(MU))
QS = 127.0 / LQ

_cached = {}


def _build_program(n_steps, warm=WARM):
    import concourse.mybir as mybir
    import concourse.tile as tile
    from concourse import bacc
    from concourse.bass import AP
    from concourse.masks import make_identity

    fp32 = mybir.dt.float32
    f32r = mybir.dt.float32r  # same bits as fp32; single-pass PE matmul mode
    bf16 = mybir.dt.bfloat16
    int8 = mybir.dt.int8
    out_dt = int8 if OUT_I8 else bf16
    Sig = mybir.ActivationFunctionType.Sigmoid
    Tanh = mybir.ActivationFunctionType.Tanh
    Ln = mybir.ActivationFunctionType.Ln
    mult = mybir.AluOpType.mult
    add = mybir.AluOpType.add
    sub = mybir.AluOpType.subtract
    is_ge = mybir.AluOpType.is_ge
    abs_max = mybir.AluOpType.abs_max

    nc = bacc.Bacc("TRN2", target_bir_lowering=False, debug=False)

    n_keep = n_steps - warm
    xp_d = nc.dram_tensor("xpool", [128, POOL * 64], bf16, kind="ExternalInput")
    dram = {}
    for s in range(N_STR):
        dram[f"rho{s}"] = nc.dram_tensor(
            f"rho{s}", [128, n_steps], fp32, kind="ExternalInput"
        )
        dram[f"whh{s}"] = nc.dram_tensor(
            f"whh{s}", [128, 512], bf16, kind="ExternalInput"
        )
        dram[f"wih{s}"] = nc.dram_tensor(
            f"wih{s}", [128, 512], bf16, kind="ExternalInput"
        )
        dram[f"wd{s}"] = nc.dram_tensor(
            f"wd{s}", [128, 128], bf16, kind="ExternalInput"
        )
        dram[f"bg{s}"] = nc.dram_tensor(
            f"bg{s}", [1, 512], f32r, kind="ExternalInput"
        )
        dram[f"bd{s}"] = nc.dram_tensor(
            f"bd{s}", [1, 128], bf16, kind="ExternalInput"
        )
        dram[f"hs{s}"] = nc.dram_tensor(
            f"hs{s}", [128, n_keep * 128], out_dt, kind="ExternalOutput"
        )

    with tile.TileContext(nc) as tc:
        with (
            tc.tile_pool(name="const", bufs=1) as cpool,
            tc.tile_pool(name="xin", bufs=2) as xpool,
            tc.tile_pool(name="outs", bufs=2) as opool,
            tc.tile_pool(name="work", bufs=3) as wpool,
            tc.tile_pool(name="state", bufs=2) as spool,
            tc.tile_pool(name="psum", bufs=2, space="PSUM") as ppool,
        ):
            identf = cpool.tile([128, 128], fp32)
            make_identity(nc, identf)
            ident = cpool.tile([128, 128], f32r)
            nc.vector.tensor_copy(ident, identf)
            ident16 = cpool.tile([128, 128], bf16)
            nc.vector.tensor_copy(ident16, identf)

            st = []  # per-stream mutable state
            for s in range(N_STR):
                whh = cpool.tile([128, 512], bf16, name=f"whh_sb{s}")
                nc.sync.dma_start(out=whh, in_=dram[f"whh{s}"][:])
                wih = cpool.tile([128, 512], bf16, name=f"wih_sb{s}")
                nc.sync.dma_start(out=wih, in_=dram[f"wih{s}"][:])
                wd = cpool.tile([128, 128], bf16, name=f"wd_sb{s}")
                nc.sync.dma_start(out=wd, in_=dram[f"wd{s}"][:])
                bg_row = cpool.tile([1, 512], f32r, name=f"bg_row{s}")
                nc.sync.dma_start(out=bg_row, in_=dram[f"bg{s}"][:])
                bias_g = cpool.tile([128, 512], f32r, name=f"bg_sb{s}")
                nc.gpsimd.partition_broadcast(bias_g, bg_row)
                bd_row = cpool.tile([1, 128], bf16, name=f"bd_row{s}")
                nc.sync.dma_start(out=bd_row, in_=dram[f"bd{s}"][:])
                bias_d = cpool.tile([128, 128], bf16, name=f"bd_sb{s}")
                nc.gpsimd.partition_broadcast(bias_d, bd_row)
                rho = cpool.tile([128, n_steps], fp32, name=f"rho_sb{s}")
                nc.sync.dma_start(out=rho, in_=dram[f"rho{s}"][:])
                sT_f = cpool.tile([128, 256], bf16, name=f"sT_i{s}")
                nc.gpsimd.memset(sT_f, 0.0)
                cbm = cpool.tile([128, 128], fp32, name=f"cbm_i{s}")
                nc.gpsimd.memset(cbm, 0.0)
                st.append({"rho": rho, "whh": whh, "wih": wih, "wd": wd,
                           "bias_g": bias_g, "bias_d": bias_d,
                           "cT": sT_f[:, 0:128], "hT": sT_f[:, 128:256],
                           "cbm": cbm, "x": None, "stg": None})

            xp_ap = xp_d[:]
            xrow = xp_ap.ap[0][0]

            def x_chunk_dma(out_tile, s, t0):
                """Assemble CH paired-window stationaries from the pool.
                F (s=0): step t uses blocks (t, t+128), ascending.
                R (s=1): step t uses blocks (175-t, 303-t); chunk stored
                ascending by block, step index reversed at use site."""
                if s == 0:
                    p0 = t0
                else:
                    p0 = (n_steps + warm) - t0 - CH
                sb = out_tile[:, 0:64]
                sbrow = sb.ap[0][0]
                for j in range(2):
                    src = AP(xp_ap.tensor, (p0 + 128 * j) * 64,
                             [[xrow, 128], [64, CH], [1, 64]])
                    dst = AP(sb.tensor, sb.offset + 64 * j,
                             [[sbrow, 128], [128, CH], [1, 64]])
                    nc.sync.dma_start(out=dst, in_=src)

            for t in range(n_steps):
                for s in range(N_STR):
                    v = st[s]
                    if t % CH == 0:
                        v["x"] = xpool.tile(
                            [128, CH * 128], bf16, tag=f"x{s}", name=f"x{s}"
                        )
                        x_chunk_dma(v["x"], s, t)
                    if t % STG == 0:
                        v["stg"] = opool.tile(
                            [128, STG * 128], fp32, tag=f"stg{s}", name=f"stg{s}"
                        )
                    ci = t % CH if s == 0 else CH - 1 - (t % CH)
                    xs = v["x"][:, ci * 128 : (ci + 1) * 128]

                    m_ps = ppool.tile(
                        [128, 384], fp32, tag=f"mix{s}", name=f"mps{s}"
                    )  # [cs | cT' | hT']
                    g_ps = ppool.tile(
                        [128, 512], fp32, tag=f"gates{s}", name=f"gps{s}"
                    )
                    # c-path first: its consumers overlap the h-matmul
                    nc.tensor.matmul(
                        m_ps[:, 0:128], ident16, v["bias_d"],
                        start=True, stop=False,
                    )
                    nc.tensor.matmul(
                        m_ps[:, 0:128], v["cT"], v["wd"],
                        start=False, stop=True, skip_group_check=True,
                    )
                    nc.tensor.matmul(
                        g_ps[:, 0:512], ident, v["bias_g"],
                        start=True, stop=False,
                    )
                    nc.tensor.matmul(
                        g_ps[:, 0:512], xs, v["wih"],
                        start=False, stop=False, skip_group_check=True,
                    )
                    nc.tensor.matmul(
                        g_ps[:, 0:512], v["hT"], v["whh"],
                        start=False, stop=True,
                    )

                    tcs = wpool.tile([128, 128], fp32, tag=f"tcs{s}", name=f"tcs{s}")
                    nc.scalar.activation(tcs, m_ps[:, 0:128], Tanh)
                    tg = wpool.tile([128, 128], fp32, tag=f"tg{s}", name=f"tg{s}")
                    nc.scalar.activation(tg, g_ps[:, 384:512], Tanh)
                    sif = wpool.tile([128, 384], fp32, tag=f"sif{s}", name=f"sif{s}")
                    nc.scalar.activation(sif, g_ps[:, 0:384], Sig)
                    so = sif[:, 256:384]

                    q1 = wpool.tile([128, 128], fp32, tag=f"q1{s}", name=f"q1{s}")
                    nc.vector.tensor_scalar(
                        q1, tcs, v["rho"][:, t : t + 1], None, mult
                    )
                    cadj = wpool.tile([128, 128], fp32, tag=f"cadj{s}", name=f"cadj{s}")
                    nc.gpsimd.tensor_tensor(cadj, v["cbm"], q1, add)
                    v1 = wpool.tile([128, 128], fp32, tag=f"v1{s}", name=f"v1{s}")
                    nc.gpsimd.tensor_tensor(v1, sif[:, 0:128], tg, mult)
                    v2 = wpool.tile([128, 128], fp32, tag=f"v2{s}", name=f"v2{s}")
                    nc.vector.tensor_tensor(v2, sif[:, 128:256], cadj, mult)
                    cbm = spool.tile([128, 128], fp32, tag=f"cbm{s}", name=f"cbm{s}")
                    nc.vector.tensor_tensor(cbm, v2, v1, add)
                    v["cbm"] = cbm
                    tcn = wpool.tile([128, 128], fp32, tag=f"tcn{s}", name=f"tcn{s}")
                    nc.scalar.activation(tcn, cbm, Tanh)
                    hs_f = v["stg"][:, (t % STG) * 128 : (t % STG + 1) * 128]
                    nc.vector.tensor_tensor(hs_f, so, tcn, mult)

                    nc.tensor.transpose(m_ps[:, 128:256], cbm, identf)
                    nc.tensor.transpose(m_ps[:, 256:384], hs_f, identf)
                    stT = spool.tile([128, 256], bf16, tag=f"stT{s}", name=f"stT{s}")
                    nc.vector.tensor_copy(stT, m_ps[:, 128:384])
                    v["cT"] = stT[:, 0:128]
                    v["hT"] = stT[:, 128:256]

                    if t >= warm and t % STG == STG - 1:
                        # flush: encode the whole 8-step block wide, then DMA
                        t0 = ((t - warm) // STG) * STG  # dram offset, kept steps
                        W = STG * 128
                        blk = v["stg"][:, 0:W]
                        if OUT_I8:
                            # mu-law int8: q = round(QS*ln(1+MU*|h|)) * sign(h)
                            s05 = wpool.tile([128, W], fp32,
                                             tag=f"s05{s}", name=f"s05{s}")
                            nc.vector.tensor_scalar(
                                s05, blk, 0.0, 0.5, is_ge, sub
                            )  # +-0.5 by sign of h
                            ab = wpool.tile([128, W], fp32,
                                            tag=f"ab{s}", name=f"ab{s}")
                            nc.vector.scalar_tensor_tensor(
                                ab, blk, 2.0 * MU, s05, mult, mult
                            )  # MU*|h|
                            ll = wpool.tile([128, W], fp32,
                                            tag=f"ll{s}", name=f"ll{s}")
                            nc.scalar.activation(ll, ab, Ln, bias=1.0)
                            q8 = opool.tile([128, W], out_dt,
                                            tag=f"q8{s}", name=f"q8{s}")
                            # (ll * 2QS) * (+-0.5); int8 convert rounds-to-nearest
                            nc.vector.scalar_tensor_tensor(
                                q8, ll, 2.0 * QS, s05, mult, mult
                            )
                        else:
                            q8 = opool.tile([128, W], out_dt,
                                            tag=f"q8{s}", name=f"q8{s}")
                            nc.gpsimd.tensor_copy(q8, blk)
                        nc.sync.dma_start(
                            out=dram[f"hs{s}"][:, t0 * 128 : t0 * 128 + W],
                            in_=q8,
                        )

    nc.compile()
    return nc


def _get_program(n_steps):
    if n_steps not in _cached:
        _cached[n_steps] = _build_program(n_steps)
    return _cached[n_steps]


_PERM = np.concatenate(
    [np.arange(0, 128), np.arange(128, 256), np.arange(384, 512), np.arange(256, 384)]
)  # reference gate order [i,f,g,o] -> kernel order [i,f,o,g]


def _sigmoid(z):
    return 1.0 / (1.0 + np.exp(-z))


def _host_scan(x_seq, dt_seq, h, c, Wih, Whh, bihh, Wd, bd):
    """Exact reference TLSTM steps on host (numpy fp32). x_seq: [T,B,I],
    dt_seq: [T,B]. Returns ys [T,B,H]."""
    T = x_seq.shape[0]
    ys = np.empty((T, x_seq.shape[1], Wd.shape[0]), np.float32)
    for t in range(T):
        c_s = np.tanh(c @ Wd.T + bd)
        c_adj = c - c_s + c_s / np.log(E + dt_seq[t][:, None])
        gates = x_seq[t] @ Wih.T + bihh + h @ Whh.T
        i_g, f_g, g_g, o_g = np.split(gates, 4, axis=-1)
        c = _sigmoid(f_g) * c_adj + _sigmoid(i_g) * np.tanh(g_g)
        h = _sigmoid(o_g) * np.tanh(c)
        ys[t] = h
    return ys


def _stream_rho(dt_dir, wA, wB):
    """rho tile [128, L] for a stream packing windows (rows 0-63 = wA)."""
    rho = np.zeros((128, L), np.float32)
    for j, w in enumerate((wA, wB)):
        t0 = w * L_KEEP - WARM
        sl = slice(64 * j, 64 * (j + 1))
        lo = max(0, -t0)           # pad steps at the head (window 0 only)
        hi = min(L, dt_dir.shape[0] - t0)
        if hi > lo:
            r = 1.0 / np.log(E + dt_dir[t0 + lo : t0 + hi])  # [n, B]
            rho[sl, lo:hi] = (r - 1.0).T
    return rho


def kernel(**inputs):
    from concourse.bass_utils import run_bass_kernel_spmd

    x = np.asarray(inputs["x"], np.float32)
    h0 = np.asarray(inputs["h0"], np.float32)
    c0 = np.asarray(inputs["c0"], np.float32)
    dt_sb = np.asarray(inputs["delta_ts"], np.float32).T  # [S, B]

    wsets = []
    for dsuf in ("f", "r"):
        Wih = np.asarray(inputs[f"W_ih_{dsuf}"], np.float32)
        Whh = np.asarray(inputs[f"W_hh_{dsuf}"], np.float32)
        bihh = (
            np.asarray(inputs[f"b_ih_{dsuf}"], np.float32)
            + np.asarray(inputs[f"b_hh_{dsuf}"], np.float32)
        )
        Wd = np.asarray(inputs[f"W_d_{dsuf}"], np.float32)
        bd = np.asarray(inputs[f"b_d_{dsuf}"], np.float32)
        wsets.append((Wih, Whh, bihh, Wd, bd))

    dt_dirs = [dt_sb, dt_sb[::-1]]

    # x pool, feature-major: xT padded to [I, S + 2*WARM] then per-core slices
    xT = np.zeros((I, S + 2 * WARM, B), BF16)
    xT[:, WARM : WARM + S] = x.transpose(2, 0, 1)

    nc = _get_program(L)

    in_maps = []
    meta = []
    for core in range(NCORES):
        j = core
        pool = xT[:, 256 * j : 256 * j + POOL]  # already WARM-shifted by pad
        m = {"xpool": np.ascontiguousarray(pool.reshape(128, POOL * 64))}
        # stream 0: forward windows (2j, 2j+1); stream 1: reverse (15-2j, 14-2j)
        wins = [(2 * j, 2 * j + 1), (15 - 2 * j, 14 - 2 * j)]
        for s, d in enumerate((0, 1)):
            Wih, Whh, bihh, Wd, bd = wsets[d]
            m[f"wih{s}"] = np.ascontiguousarray(Wih[_PERM].T).astype(BF16)
            m[f"whh{s}"] = np.ascontiguousarray(Whh[_PERM].T).astype(BF16)
            m[f"wd{s}"] = np.ascontiguousarray(Wd.T).astype(BF16)
            m[f"bg{s}"] = np.ascontiguousarray(bihh[_PERM][None, :])
            m[f"bd{s}"] = np.ascontiguousarray(bd[None, :]).astype(BF16)
            m[f"rho{s}"] = _stream_rho(dt_dirs[d], *wins[s])
        in_maps.append(m)
        meta.append(wins)

    global _last_in_maps
    _last_in_maps = in_maps
    res = run_bass_kernel_spmd(nc, in_maps, list(range(NCORES)))

    out = np.empty((S, B, 2 * H), np.float32)
    for core in range(NCORES):
        wins = meta[core]
        for s, d in enumerate((0, 1)):
            raw = res.results[core][f"hs{s}"]
            if OUT_I8:
                q = np.asarray(raw, np.float32)
                hsd = np.sign(q) * np.expm1(np.abs(q) / QS) / MU
            else:
                hsd = np.asarray(raw, np.float32)
            hs = hsd.reshape(128, L_KEEP, 128)
            for j, w in enumerate(wins[s]):
                ys = hs[64 * j : 64 * (j + 1)].transpose(1, 0, 2)  # [KEEP, B, H]
                p_lo = w * L_KEEP  # kept positions, direction-local
                if d == 0:
                    out[p_lo : p_lo + L_KEEP, :, 0:H] = ys
                else:
                    orig_hi = S - 1 - p_lo
                    orig_lo = S - 1 - (p_lo + L_KEEP)
                    out[orig_hi : None if orig_lo < 0 else orig_lo : -1,
                        :, H : 2 * H] = ys
    # host fixup: first WARM kept steps of window 0, each direction (exact;
    # these are the only outputs that truly depend on h0/c0)
    x_dirs = [x, x[::-1]]
    for d in range(2):
        Wih, Whh, bihh, Wd, bd = wsets[d]
        ys = _host_scan(
            x_dirs[d][0:WARM], dt_dirs[d][0:WARM],
            h0[d].copy(), c0[d].copy(), Wih, Whh, bihh, Wd, bd
        )
        if d == 0:
            out[0:WARM, :, 0:H] = ys
        else:
            out[S - 1 : S - 1 - WARM : -1, :, H : 2 * H] = ys
    return out


# revision 15
# speedup vs baseline: 1.5600x; 1.5600x over previous
"""Bidirectional Time-aware LSTM (TLSTM) for Trainium2, 8 NeuronCores.

Strategy: sequence-chunked parallelism, sharded by SEQUENCE position so the
two directions share one x upload. Core j owns positions [256j, 256j+256):
its F-stream runs forward windows (2j, 2j+1), its R-stream runs reverse
windows (15-2j, 14-2j) - both read the same on-device x pool of 304
positions ([256j-24, 256j+280), zero-padded outside [0, 2048)).

Windows are recomputed exactly by starting WARM=24 steps early from zero
state (the forget gate contracts state error ~0.6/step, so 24 steps reach
well below the bf16 noise floor). All 16 windows per direction use the
uniform start t0(w) = 128w - WARM; window 0 warms up over zero-padded
positions and converges onto the true trajectory, and the first WARM kept
positions of each direction are recomputed exactly on the host (they are
the only outputs that genuinely depend on h0/c0).

Host->device traffic dominates the cost, so the kernel ships raw x once
(bf16, feature-major pool blocks of [128 x 64]) and computes x @ W_ih^T
on-device each step with the step's x as the PE stationary operand; each
stream's paired-window stationary tiles are assembled from the shared pool
by strided on-device DMA. Biases ship as [1, n] rows and are partition-
broadcast on device. Outputs ship back as mu-law companded int8
(q = round(127 * ln(1+MU*|h|)/ln(1+MU)) * sign(h)), kept steps only;
the host decodes exactly.

Per stream per step (batch-major rows = 2x64 batch):
  psum_m[:, :128] = b_d (bcast const) + c @ W_d^T          (bf16 path)
  psum_g[:, :512] = b (bcast const) + xT_t stationary @ W_ih^T + h @ W_hh^T
  c_s = tanh(psum_m); sig_if/sig_o = sigmoid(gates); tg = tanh(g-block)
  c' = sig_f*(c + c_s*(r-1)) + sig_i*tg ;  h' = sig_o*tanh(c')
  c'/h' transposed on PE back to feature-major (bf16) for the next step.
"""

import os
import sys

import numpy as np
import ml_dtypes

for _p in ("/opt/trn_rl_repo",):
    if _p not in sys.path and os.path.isdir(_p):
        sys.path.insert(0, _p)

BF16 = ml_dtypes.bfloat16

S, B, I, H = 2048, 64, 128, 128
E = float(np.e)
NCORES = 8
N_STR = 2                     # streams per core: 0 = forward, 1 = reverse
N_WIN = 16                    # windows per direction
L_KEEP = S // N_WIN           # 128 kept steps per window
WARM = int(os.environ.get('TL_WARM', '24'))
L = L_KEEP + WARM             # wall steps per stream
POOL = 2 * L_KEEP + 2 * WARM  # x pool positions per core (304)
CH = 8                        # x-chunk steps per input DMA
STG = 8                       # output staging steps per output DMA

OUT_I8 = os.environ.get('TL_OUT', 'i8') == 'i8'
ALPHA = 3.0  # tanh-compander gain: q = round(127*tanh(ALPHA*h))

_cached = {}


def _build_program(n_steps, warm=WARM):
    import concourse.mybir as mybir
    import concourse.tile as tile
    from concourse import bacc
    from concourse.bass import AP
    from concourse.masks import make_identity

    fp32 = mybir.dt.float32
    f32r = mybir.dt.float32r  # same bits as fp32; single-pass PE matmul mode
    bf16 = mybir.dt.bfloat16
    int8 = mybir.dt.int8
    out_dt = int8 if OUT_I8 else bf16
    Sig = mybir.ActivationFunctionType.Sigmoid
    Tanh = mybir.ActivationFunctionType.Tanh
    Ln = mybir.ActivationFunctionType.Ln
    mult = mybir.AluOpType.mult
    add = mybir.AluOpType.add
    sub = mybir.AluOpType.subtract
    is_ge = mybir.AluOpType.is_ge
    abs_max = mybir.AluOpType.abs_max

    nc = bacc.Bacc("TRN2", target_bir_lowering=False, debug=False)

    n_keep = n_steps - warm
    xp_d = nc.dram_tensor("xpool", [128, POOL * 64], bf16, kind="ExternalInput")
    dram = {}
    for s in range(N_STR):
        dram[f"rho{s}"] = nc.dram_tensor(
            f"rho{s}", [128, n_steps], fp32, kind="ExternalInput"
        )
        dram[f"whh{s}"] = nc.dram_tensor(
            f"whh{s}", [128, 512], bf16, kind="ExternalInput"
        )
        dram[f"wih{s}"] = nc.dram_tensor(
            f"wih{s}", [128, 512], bf16, kind="ExternalInput"
        )
        dram[f"wd{s}"] = nc.dram_tensor(
            f"wd{s}", [128, 128], bf16, kind="ExternalInput"
        )
        dram[f"bg{s}"] = nc.dram_tensor(
            f"bg{s}", [1, 512], f32r, kind="ExternalInput"
        )
        dram[f"bd{s}"] = nc.dram_tensor(
            f"bd{s}", [1, 128], bf16, kind="ExternalInput"
        )
        dram[f"hs{s}"] = nc.dram_tensor(
            f"hs{s}", [128, n_keep * 128], out_dt, kind="ExternalOutput"
        )

    with tile.TileContext(nc) as tc:
        with (
            tc.tile_pool(name="const", bufs=1) as cpool,
            tc.tile_pool(name="xin", bufs=2) as xpool,
            tc.tile_pool(name="outs", bufs=2) as opool,
            tc.tile_pool(name="work", bufs=3) as wpool,
            tc.tile_pool(name="state", bufs=2) as spool,
            tc.tile_pool(name="psum", bufs=2, space="PSUM") as ppool,
        ):
            identf = cpool.tile([128, 128], fp32)
            make_identity(nc, identf)
            ident = cpool.tile([128, 128], f32r)
            nc.vector.tensor_copy(ident, identf)
            ident16 = cpool.tile([128, 128], bf16)
            nc.vector.tensor_copy(ident16, identf)

            st = []  # per-stream mutable state
            for s in range(N_STR):
                whh = cpool.tile([128, 512], bf16, name=f"whh_sb{s}")
                nc.sync.dma_start(out=whh, in_=dram[f"whh{s}"][:])
                wih = cpool.tile([128, 512], bf16, name=f"wih_sb{s}")
                nc.sync.dma_start(out=wih, in_=dram[f"wih{s}"][:])
                wd = cpool.tile([128, 128], bf16, name=f"wd_sb{s}")
                nc.sync.dma_start(out=wd, in_=dram[f"wd{s}"][:])
                bg_row = cpool.tile([1, 512], f32r, name=f"bg_row{s}")
                nc.sync.dma_start(out=bg_row, in_=dram[f"bg{s}"][:])
                bias_g = cpool.tile([128, 512], f32r, name=f"bg_sb{s}")
                nc.gpsimd.partition_broadcast(bias_g, bg_row)
                bd_row = cpool.tile([1, 128], bf16, name=f"bd_row{s}")
                nc.sync.dma_start(out=bd_row, in_=dram[f"bd{s}"][:])
                bias_d = cpool.tile([128, 128], bf16, name=f"bd_sb{s}")
                nc.gpsimd.partition_broadcast(bias_d, bd_row)
                rho = cpool.tile([128, n_steps], fp32, name=f"rho_sb{s}")
                nc.sync.dma_start(out=rho, in_=dram[f"rho{s}"][:])
                sT_f = cpool.tile([128, 256], bf16, name=f"sT_i{s}")
                nc.gpsimd.memset(sT_f, 0.0)
                cbm = cpool.tile([128, 128], fp32, name=f"cbm_i{s}")
                nc.gpsimd.memset(cbm, 0.0)
                st.append({"rho": rho, "whh": whh, "wih": wih, "wd": wd,
                           "bias_g": bias_g, "bias_d": bias_d,
                           "cT": sT_f[:, 0:128], "hT": sT_f[:, 128:256],
                           "cbm": cbm, "x": None, "stg": None})

            xp_ap = xp_d[:]
            xrow = xp_ap.ap[0][0]

            def x_chunk_dma(out_tile, s, t0):
                """Assemble CH paired-window stationaries from the pool.
                F (s=0): step t uses blocks (t, t+128), ascending.
                R (s=1): step t uses blocks (175-t, 303-t); chunk stored
                ascending by block, step index reversed at use site."""
                if s == 0:
                    p0 = t0
                else:
                    p0 = (n_steps + warm) - t0 - CH
                sb = out_tile[:, 0:64]
                sbrow = sb.ap[0][0]
                for j in range(2):
                    src = AP(xp_ap.tensor, (p0 + 128 * j) * 64,
                             [[xrow, 128], [64, CH], [1, 64]])
                    dst = AP(sb.tensor, sb.offset + 64 * j,
                             [[sbrow, 128], [128, CH], [1, 64]])
                    nc.sync.dma_start(out=dst, in_=src)

            for t in range(n_steps):
                for s in range(N_STR):
                    v = st[s]
                    if t % CH == 0:
                        v["x"] = xpool.tile(
                            [128, CH * 128], bf16, tag=f"x{s}", name=f"x{s}"
                        )
                        x_chunk_dma(v["x"], s, t)
                    if t % STG == 0:
                        v["stg"] = opool.tile(
                            [128, STG * 128], fp32, tag=f"stg{s}", name=f"stg{s}"
                        )
                    ci = t % CH if s == 0 else CH - 1 - (t % CH)
                    xs = v["x"][:, ci * 128 : (ci + 1) * 128]

                    m_ps = ppool.tile(
                        [128, 384], fp32, tag=f"mix{s}", name=f"mps{s}"
                    )  # [cs | cT' | hT']
                    g_ps = ppool.tile(
                        [128, 512], fp32, tag=f"gates{s}", name=f"gps{s}"
                    )
                    # c-path first: its consumers overlap the h-matmul
                    nc.tensor.matmul(
                        m_ps[:, 0:128], ident16, v["bias_d"],
                        start=True, stop=False,
                    )
                    nc.tensor.matmul(
                        m_ps[:, 0:128], v["cT"], v["wd"],
                        start=False, stop=True, skip_group_check=True,
                    )
                    nc.tensor.matmul(
                        g_ps[:, 0:512], ident, v["bias_g"],
                        start=True, stop=False,
                    )
                    nc.tensor.matmul(
                        g_ps[:, 0:512], xs, v["wih"],
                        start=False, stop=False, skip_group_check=True,
                    )
                    nc.tensor.matmul(
                        g_ps[:, 0:512], v["hT"], v["whh"],
                        start=False, stop=True,
                    )

                    tcs = wpool.tile([128, 128], fp32, tag=f"tcs{s}", name=f"tcs{s}")
                    nc.scalar.activation(tcs, m_ps[:, 0:128], Tanh)
                    tg = wpool.tile([128, 128], fp32, tag=f"tg{s}", name=f"tg{s}")
                    nc.scalar.activation(tg, g_ps[:, 384:512], Tanh)
                    sif = wpool.tile([128, 384], fp32, tag=f"sif{s}", name=f"sif{s}")
                    nc.scalar.activation(sif, g_ps[:, 0:384], Sig)
                    so = sif[:, 256:384]

                    q1 = wpool.tile([128, 128], fp32, tag=f"q1{s}", name=f"q1{s}")
                    nc.vector.tensor_scalar(
                        q1, tcs, v["rho"][:, t : t + 1], None, mult
                    )
                    cadj = wpool.tile([128, 128], fp32, tag=f"cadj{s}", name=f"cadj{s}")
                    nc.gpsimd.tensor_tensor(cadj, v["cbm"], q1, add)
                    v1 = wpool.tile([128, 128], fp32, tag=f"v1{s}", name=f"v1{s}")
                    nc.gpsimd.tensor_tensor(v1, sif[:, 0:128], tg, mult)
                    v2 = wpool.tile([128, 128], fp32, tag=f"v2{s}", name=f"v2{s}")
                    nc.vector.tensor_tensor(v2, sif[:, 128:256], cadj, mult)
                    cbm = spool.tile([128, 128], fp32, tag=f"cbm{s}", name=f"cbm{s}")
                    nc.vector.tensor_tensor(cbm, v2, v1, add)
                    v["cbm"] = cbm
                    tcn = wpool.tile([128, 128], fp32, tag=f"tcn{s}", name=f"tcn{s}")
                    nc.scalar.activation(tcn, cbm, Tanh)
                    hs_f = v["stg"][:, (t % STG) * 128 : (t % STG + 1) * 128]
                    nc.vector.tensor_tensor(hs_f, so, tcn, mult)

                    nc.tensor.transpose(m_ps[:, 128:256], cbm, identf)
                    nc.tensor.transpose(m_ps[:, 256:384], hs_f, identf)
                    stT = spool.tile([128, 256], bf16, tag=f"stT{s}", name=f"stT{s}")
                    nc.vector.tensor_copy(stT, m_ps[:, 128:384])
                    v["cT"] = stT[:, 0:128]
                    v["hT"] = stT[:, 128:256]

                    if t >= warm and t % STG == STG - 1:
                        # flush: encode the whole 8-step block wide, then DMA
                        t0 = ((t - warm) // STG) * STG  # dram offset, kept steps
                        W = STG * 128
                        blk = v["stg"][:, 0:W]
                        if OUT_I8:
                            # tanh-compander int8: q = round(127*tanh(ALPHA*h))
                            # (odd function: no abs/sign needed; Tanh table is
                            # already resident - no act-table thrash)
                            ll = wpool.tile([128, W], fp32,
                                            tag=f"ll{s}", name=f"ll{s}")
                            nc.scalar.activation(ll, blk, Tanh, scale=ALPHA)
                            q8 = opool.tile([128, W], out_dt,
                                            tag=f"q8{s}", name=f"q8{s}")
                            # int8 convert rounds-to-nearest-even
                            nc.vector.tensor_scalar(
                                q8, ll, 127.0, None, mult
                            )
                        else:
                            q8 = opool.tile([128, W], out_dt,
                                            tag=f"q8{s}", name=f"q8{s}")
                            nc.gpsimd.tensor_copy(q8, blk)
                        nc.sync.dma_start(
                            out=dram[f"hs{s}"][:, t0 * 128 : t0 * 128 + W],
                            in_=q8,
                        )

    nc.compile()
    return nc


def _get_program(n_steps):
    if n_steps not in _cached:
        _cached[n_steps] = _build_program(n_steps)
    return _cached[n_steps]


_PERM = np.concatenate(
    [np.arange(0, 128), np.arange(128, 256), np.arange(384, 512), np.arange(256, 384)]
)  # reference gate order [i,f,g,o] -> kernel order [i,f,o,g]


def _sigmoid(z):
    return 1.0 / (1.0 + np.exp(-z))


def _host_scan(x_seq, dt_seq, h, c, Wih, Whh, bihh, Wd, bd):
    """Exact reference TLSTM steps on host (numpy fp32). x_seq: [T,B,I],
    dt_seq: [T,B]. Returns ys [T,B,H]."""
    T = x_seq.shape[0]
    ys = np.empty((T, x_seq.shape[1], Wd.shape[0]), np.float32)
    for t in range(T):
        c_s = np.tanh(c @ Wd.T + bd)
        c_adj = c - c_s + c_s / np.log(E + dt_seq[t][:, None])
        gates = x_seq[t] @ Wih.T + bihh + h @ Whh.T
        i_g, f_g, g_g, o_g = np.split(gates, 4, axis=-1)
        c = _sigmoid(f_g) * c_adj + _sigmoid(i_g) * np.tanh(g_g)
        h = _sigmoid(o_g) * np.tanh(c)
        ys[t] = h
    return ys


def _stream_rho(dt_dir, wA, wB):
    """rho tile [128, L] for a stream packing windows (rows 0-63 = wA)."""
    rho = np.zeros((128, L), np.float32)
    for j, w in enumerate((wA, wB)):
        t0 = w * L_KEEP - WARM
        sl = slice(64 * j, 64 * (j + 1))
        lo = max(0, -t0)           # pad steps at the head (window 0 only)
        hi = min(L, dt_dir.shape[0] - t0)
        if hi > lo:
            r = 1.0 / np.log(E + dt_dir[t0 + lo : t0 + hi])  # [n, B]
            rho[sl, lo:hi] = (r - 1.0).T
    return rho


def kernel(**inputs):
    from concourse.bass_utils import run_bass_kernel_spmd

    x = np.asarray(inputs["x"], np.float32)
    h0 = np.asarray(inputs["h0"], np.float32)
    c0 = np.asarray(inputs["c0"], np.float32)
    dt_sb = np.asarray(inputs["delta_ts"], np.float32).T  # [S, B]

    wsets = []
    for dsuf in ("f", "r"):
        Wih = np.asarray(inputs[f"W_ih_{dsuf}"], np.float32)
        Whh = np.asarray(inputs[f"W_hh_{dsuf}"], np.float32)
        bihh = (
            np.asarray(inputs[f"b_ih_{dsuf}"], np.float32)
            + np.asarray(inputs[f"b_hh_{dsuf}"], np.float32)
        )
        Wd = np.asarray(inputs[f"W_d_{dsuf}"], np.float32)
        bd = np.asarray(inputs[f"b_d_{dsuf}"], np.float32)
        wsets.append((Wih, Whh, bihh, Wd, bd))

    dt_dirs = [dt_sb, dt_sb[::-1]]

    # x pool, feature-major: xT padded to [I, S + 2*WARM] then per-core slices
    xT = np.zeros((I, S + 2 * WARM, B), BF16)
    xT[:, WARM : WARM + S] = x.transpose(2, 0, 1)

    nc = _get_program(L)

    in_maps = []
    meta = []
    for core in range(NCORES):
        j = core
        pool = xT[:, 256 * j : 256 * j + POOL]  # already WARM-shifted by pad
        m = {"xpool": np.ascontiguousarray(pool.reshape(128, POOL * 64))}
        # stream 0: forward windows (2j, 2j+1); stream 1: reverse (15-2j, 14-2j)
        wins = [(2 * j, 2 * j + 1), (15 - 2 * j, 14 - 2 * j)]
        for s, d in enumerate((0, 1)):
            Wih, Whh, bihh, Wd, bd = wsets[d]
            m[f"wih{s}"] = np.ascontiguousarray(Wih[_PERM].T).astype(BF16)
            m[f"whh{s}"] = np.ascontiguousarray(Whh[_PERM].T).astype(BF16)
            m[f"wd{s}"] = np.ascontiguousarray(Wd.T).astype(BF16)
            m[f"bg{s}"] = np.ascontiguousarray(bihh[_PERM][None, :])
            m[f"bd{s}"] = np.ascontiguousarray(bd[None, :]).astype(BF16)
            m[f"rho{s}"] = _stream_rho(dt_dirs[d], *wins[s])
        in_maps.append(m)
        meta.append(wins)

    global _last_in_maps
    _last_in_maps = in_maps
    res = run_bass_kernel_spmd(nc, in_maps, list(range(NCORES)))

    out = np.empty((S, B, 2 * H), np.float32)
    for core in range(NCORES):
        wins = meta[core]
        for s, d in enumerate((0, 1)):
            raw = res.results[core][f"hs{s}"]
            if OUT_I8:
                q = np.asarray(raw, np.float32)
                hsd = np.arctanh(
                    np.clip(q / 127.0, -126.9 / 127.0, 126.9 / 127.0)
                ) / ALPHA
            else:
                hsd = np.asarray(raw, np.float32)
            hs = hsd.reshape(128, L_KEEP, 128)
            for j, w in enumerate(wins[s]):
                ys = hs[64 * j : 64 * (j + 1)].transpose(1, 0, 2)  # [KEEP, B, H]
                p_lo = w * L_KEEP  # kept positions, direction-local
                if d == 0:
                    out[p_lo : p_lo + L_KEEP, :, 0:H] = ys
                else:
                    orig_hi = S - 1 - p_lo
                    orig_lo = S - 1 - (p_lo + L_KEEP)
                    out[orig_hi : None if orig_lo < 0 else orig_lo : -1,
                        :, H : 2 * H] = ys
    # host fixup: first WARM kept steps of window 0, each direction (exact;
    # these are the only outputs that truly depend on h0/c0)
    x_dirs = [x, x[::-1]]
    for d in range(2):
        Wih, Whh, bihh, Wd, bd = wsets[d]
        ys = _host_scan(
            x_dirs[d][0:WARM], dt_dirs[d][0:WARM],
            h0[d].copy(), c0[d].copy(), Wih, Whh, bihh, Wd, bd
        )
        if d == 0:
            out[0:WARM, :, 0:H] = ys
        else:
            out[S - 1 : S - 1 - WARM : -1, :, H : 2 * H] = ys
    return out


# revision 16
# speedup vs baseline: 1.6370x; 1.0494x over previous
"""Bidirectional Time-aware LSTM (TLSTM) for Trainium2, 8 NeuronCores.

Strategy: sequence-chunked parallelism, sharded by SEQUENCE position so the
two directions share one x upload. Core j owns positions [256j, 256j+256):
its F-stream runs forward windows (2j, 2j+1), its R-stream runs reverse
windows (15-2j, 14-2j) - both read the same on-device x pool of 304
positions ([256j-24, 256j+280), zero-padded outside [0, 2048)).

Windows are recomputed exactly by starting WARM=24 steps early from zero
state (the forget gate contracts state error ~0.6/step, so 24 steps reach
well below the bf16 noise floor). All 16 windows per direction use the
uniform start t0(w) = 128w - WARM; window 0 warms up over zero-padded
positions and converges onto the true trajectory, and the first WARM kept
positions of each direction are recomputed exactly on the host (they are
the only outputs that genuinely depend on h0/c0).

Host->device traffic dominates the cost, so the kernel ships raw x once
(bf16, feature-major pool blocks of [128 x 64]) and computes x @ W_ih^T
on-device each step with the step's x as the PE stationary operand; each
stream's paired-window stationary tiles are assembled from the shared pool
by strided on-device DMA. Biases ship as [1, n] rows and are partition-
broadcast on device. Outputs ship back as mu-law companded int8
(q = round(127 * ln(1+MU*|h|)/ln(1+MU)) * sign(h)), kept steps only;
the host decodes exactly.

Per stream per step (batch-major rows = 2x64 batch):
  psum_m[:, :128] = b_d (bcast const) + c @ W_d^T          (bf16 path)
  psum_g[:, :512] = b (bcast const) + xT_t stationary @ W_ih^T + h @ W_hh^T
  c_s = tanh(psum_m); sig_if/sig_o = sigmoid(gates); tg = tanh(g-block)
  c' = sig_f*(c + c_s*(r-1)) + sig_i*tg ;  h' = sig_o*tanh(c')
  c'/h' transposed on PE back to feature-major (bf16) for the next step.
"""

import os
import sys

import numpy as np
import ml_dtypes

for _p in ("/opt/trn_rl_repo",):
    if _p not in sys.path and os.path.isdir(_p):
        sys.path.insert(0, _p)

BF16 = ml_dtypes.bfloat16

S, B, I, H = 2048, 64, 128, 128
E = float(np.e)
NCORES = 8
N_STR = 2                     # streams per core: 0 = forward, 1 = reverse
N_WIN = 16                    # windows per direction
L_KEEP = S // N_WIN           # 128 kept steps per window
WARM = int(os.environ.get('TL_WARM', '16'))
L = L_KEEP + WARM             # wall steps per stream
POOL = 2 * L_KEEP + 2 * WARM  # x pool positions per core (304)
CH = 8                        # x-chunk steps per input DMA
STG = 8                       # output staging steps per output DMA

OUT_I8 = os.environ.get('TL_OUT', 'i8') == 'i8'
ALPHA = 3.0  # tanh-compander gain: q = round(127*tanh(ALPHA*h))

_cached = {}


def _build_program(n_steps, warm=WARM):
    import concourse.mybir as mybir
    import concourse.tile as tile
    from concourse import bacc
    from concourse.bass import AP
    from concourse.masks import make_identity

    fp32 = mybir.dt.float32
    f32r = mybir.dt.float32r  # same bits as fp32; single-pass PE matmul mode
    bf16 = mybir.dt.bfloat16
    int8 = mybir.dt.int8
    out_dt = int8 if OUT_I8 else bf16
    Sig = mybir.ActivationFunctionType.Sigmoid
    Tanh = mybir.ActivationFunctionType.Tanh
    Ln = mybir.ActivationFunctionType.Ln
    mult = mybir.AluOpType.mult
    add = mybir.AluOpType.add
    sub = mybir.AluOpType.subtract
    is_ge = mybir.AluOpType.is_ge
    abs_max = mybir.AluOpType.abs_max

    nc = bacc.Bacc("TRN2", target_bir_lowering=False, debug=False)

    n_keep = n_steps - warm
    xp_d = nc.dram_tensor("xpool", [128, POOL * 64], bf16, kind="ExternalInput")
    dram = {}
    for s in range(N_STR):
        dram[f"rho{s}"] = nc.dram_tensor(
            f"rho{s}", [128, n_steps], fp32, kind="ExternalInput"
        )
        dram[f"whh{s}"] = nc.dram_tensor(
            f"whh{s}", [128, 512], bf16, kind="ExternalInput"
        )
        dram[f"wih{s}"] = nc.dram_tensor(
            f"wih{s}", [128, 512], bf16, kind="ExternalInput"
        )
        dram[f"wd{s}"] = nc.dram_tensor(
            f"wd{s}", [128, 128], bf16, kind="ExternalInput"
        )
        dram[f"bg{s}"] = nc.dram_tensor(
            f"bg{s}", [1, 512], f32r, kind="ExternalInput"
        )
        dram[f"bd{s}"] = nc.dram_tensor(
            f"bd{s}", [1, 128], bf16, kind="ExternalInput"
        )
        dram[f"hs{s}"] = nc.dram_tensor(
            f"hs{s}", [128, n_keep * 128], out_dt, kind="ExternalOutput"
        )

    with tile.TileContext(nc) as tc:
        with (
            tc.tile_pool(name="const", bufs=1) as cpool,
            tc.tile_pool(name="xin", bufs=2) as xpool,
            tc.tile_pool(name="outs", bufs=2) as opool,
            tc.tile_pool(name="work", bufs=3) as wpool,
            tc.tile_pool(name="state", bufs=2) as spool,
            tc.tile_pool(name="psum", bufs=2, space="PSUM") as ppool,
        ):
            identf = cpool.tile([128, 128], fp32)
            make_identity(nc, identf)
            ident = cpool.tile([128, 128], f32r)
            nc.vector.tensor_copy(ident, identf)
            ident16 = cpool.tile([128, 128], bf16)
            nc.vector.tensor_copy(ident16, identf)

            st = []  # per-stream mutable state
            for s in range(N_STR):
                whh = cpool.tile([128, 512], bf16, name=f"whh_sb{s}")
                nc.sync.dma_start(out=whh, in_=dram[f"whh{s}"][:])
                wih = cpool.tile([128, 512], bf16, name=f"wih_sb{s}")
                nc.sync.dma_start(out=wih, in_=dram[f"wih{s}"][:])
                wd = cpool.tile([128, 128], bf16, name=f"wd_sb{s}")
                nc.sync.dma_start(out=wd, in_=dram[f"wd{s}"][:])
                bg_row = cpool.tile([1, 512], f32r, name=f"bg_row{s}")
                nc.sync.dma_start(out=bg_row, in_=dram[f"bg{s}"][:])
                bias_g = cpool.tile([128, 512], f32r, name=f"bg_sb{s}")
                nc.gpsimd.partition_broadcast(bias_g, bg_row)
                bd_row = cpool.tile([1, 128], bf16, name=f"bd_row{s}")
                nc.sync.dma_start(out=bd_row, in_=dram[f"bd{s}"][:])
                bias_d = cpool.tile([128, 128], bf16, name=f"bd_sb{s}")
                nc.gpsimd.partition_broadcast(bias_d, bd_row)
                rho = cpool.tile([128, n_steps], fp32, name=f"rho_sb{s}")
                nc.sync.dma_start(out=rho, in_=dram[f"rho{s}"][:])
                sT_f = cpool.tile([128, 256], bf16, name=f"sT_i{s}")
                nc.gpsimd.memset(sT_f, 0.0)
                cbm = cpool.tile([128, 128], fp32, name=f"cbm_i{s}")
                nc.gpsimd.memset(cbm, 0.0)
                st.append({"rho": rho, "whh": whh, "wih": wih, "wd": wd,
                           "bias_g": bias_g, "bias_d": bias_d,
                           "cT": sT_f[:, 0:128], "hT": sT_f[:, 128:256],
                           "cbm": cbm, "x": None, "stg": None})

            xp_ap = xp_d[:]
            xrow = xp_ap.ap[0][0]

            def x_chunk_dma(out_tile, s, t0):
                """Assemble CH paired-window stationaries from the pool.
                F (s=0): step t uses blocks (t, t+128), ascending.
                R (s=1): step t uses blocks (175-t, 303-t); chunk stored
                ascending by block, step index reversed at use site."""
                if s == 0:
                    p0 = t0
                else:
                    p0 = (n_steps + warm) - t0 - CH
                sb = out_tile[:, 0:64]
                sbrow = sb.ap[0][0]
                for j in range(2):
                    src = AP(xp_ap.tensor, (p0 + 128 * j) * 64,
                             [[xrow, 128], [64, CH], [1, 64]])
                    dst = AP(sb.tensor, sb.offset + 64 * j,
                             [[sbrow, 128], [128, CH], [1, 64]])
                    nc.sync.dma_start(out=dst, in_=src)

            for t in range(n_steps):
                for s in range(N_STR):
                    v = st[s]
                    if t % CH == 0:
                        v["x"] = xpool.tile(
                            [128, CH * 128], bf16, tag=f"x{s}", name=f"x{s}"
                        )
                        x_chunk_dma(v["x"], s, t)
                    if t % STG == 0:
                        v["stg"] = opool.tile(
                            [128, STG * 128], fp32, tag=f"stg{s}", name=f"stg{s}"
                        )
                    ci = t % CH if s == 0 else CH - 1 - (t % CH)
                    xs = v["x"][:, ci * 128 : (ci + 1) * 128]

                    m_ps = ppool.tile(
                        [128, 384], fp32, tag=f"mix{s}", name=f"mps{s}"
                    )  # [cs | cT' | hT']
                    g_ps = ppool.tile(
                        [128, 512], fp32, tag=f"gates{s}", name=f"gps{s}"
                    )
                    # c-path first: its consumers overlap the h-matmul
                    nc.tensor.matmul(
                        m_ps[:, 0:128], ident16, v["bias_d"],
                        start=True, stop=False,
                    )
                    nc.tensor.matmul(
                        m_ps[:, 0:128], v["cT"], v["wd"],
                        start=False, stop=True, skip_group_check=True,
                    )
                    nc.tensor.matmul(
                        g_ps[:, 0:512], ident, v["bias_g"],
                        start=True, stop=False,
                    )
                    nc.tensor.matmul(
                        g_ps[:, 0:512], xs, v["wih"],
                        start=False, stop=False, skip_group_check=True,
                    )
                    nc.tensor.matmul(
                        g_ps[:, 0:512], v["hT"], v["whh"],
                        start=False, stop=True,
                    )

                    tcs = wpool.tile([128, 128], fp32, tag=f"tcs{s}", name=f"tcs{s}")
                    nc.scalar.activation(tcs, m_ps[:, 0:128], Tanh)
                    tg = wpool.tile([128, 128], fp32, tag=f"tg{s}", name=f"tg{s}")
                    nc.scalar.activation(tg, g_ps[:, 384:512], Tanh)
                    sif = wpool.tile([128, 384], fp32, tag=f"sif{s}", name=f"sif{s}")
                    nc.scalar.activation(sif, g_ps[:, 0:384], Sig)
                    so = sif[:, 256:384]

                    q1 = wpool.tile([128, 128], fp32, tag=f"q1{s}", name=f"q1{s}")
                    nc.vector.tensor_scalar(
                        q1, tcs, v["rho"][:, t : t + 1], None, mult
                    )
                    cadj = wpool.tile([128, 128], fp32, tag=f"cadj{s}", name=f"cadj{s}")
                    nc.gpsimd.tensor_tensor(cadj, v["cbm"], q1, add)
                    v1 = wpool.tile([128, 128], fp32, tag=f"v1{s}", name=f"v1{s}")
                    nc.gpsimd.tensor_tensor(v1, sif[:, 0:128], tg, mult)
                    v2 = wpool.tile([128, 128], fp32, tag=f"v2{s}", name=f"v2{s}")
                    nc.vector.tensor_tensor(v2, sif[:, 128:256], cadj, mult)
                    cbm = spool.tile([128, 128], fp32, tag=f"cbm{s}", name=f"cbm{s}")
                    nc.vector.tensor_tensor(cbm, v2, v1, add)
                    v["cbm"] = cbm
                    tcn = wpool.tile([128, 128], fp32, tag=f"tcn{s}", name=f"tcn{s}")
                    nc.scalar.activation(tcn, cbm, Tanh)
                    hs_f = v["stg"][:, (t % STG) * 128 : (t % STG + 1) * 128]
                    nc.vector.tensor_tensor(hs_f, so, tcn, mult)

                    nc.tensor.transpose(m_ps[:, 128:256], cbm, identf)
                    nc.tensor.transpose(m_ps[:, 256:384], hs_f, identf)
                    stT = spool.tile([128, 256], bf16, tag=f"stT{s}", name=f"stT{s}")
                    nc.vector.tensor_copy(stT, m_ps[:, 128:384])
                    v["cT"] = stT[:, 0:128]
                    v["hT"] = stT[:, 128:256]

                    if t >= warm and t % STG == STG - 1:
                        # flush: encode the whole 8-step block wide, then DMA
                        t0 = ((t - warm) // STG) * STG  # dram offset, kept steps
                        W = STG * 128
                        blk = v["stg"][:, 0:W]
                        if OUT_I8:
                            # tanh-compander int8: q = round(127*tanh(ALPHA*h))
                            # (odd function: no abs/sign needed; Tanh table is
                            # already resident - no act-table thrash)
                            ll = wpool.tile([128, W], fp32,
                                            tag=f"ll{s}", name=f"ll{s}")
                            nc.scalar.activation(ll, blk, Tanh, scale=ALPHA)
                            q8 = opool.tile([128, W], out_dt,
                                            tag=f"q8{s}", name=f"q8{s}")
                            # int8 convert rounds-to-nearest-even
                            nc.vector.tensor_scalar(
                                q8, ll, 127.0, None, mult
                            )
                        else:
                            q8 = opool.tile([128, W], out_dt,
                                            tag=f"q8{s}", name=f"q8{s}")
                            nc.gpsimd.tensor_copy(q8, blk)
                        nc.sync.dma_start(
                            out=dram[f"hs{s}"][:, t0 * 128 : t0 * 128 + W],
                            in_=q8,
                        )

    nc.compile()
    return nc


def _get_program(n_steps):
    if n_steps not in _cached:
        _cached[n_steps] = _build_program(n_steps)
    return _cached[n_steps]


_PERM = np.concatenate(
    [np.arange(0, 128), np.arange(128, 256), np.arange(384, 512), np.arange(256, 384)]
)  # reference gate order [i,f,g,o] -> kernel order [i,f,o,g]


def _sigmoid(z):
    return 1.0 / (1.0 + np.exp(-z))


def _host_scan(x_seq, dt_seq, h, c, Wih, Whh, bihh, Wd, bd):
    """Exact reference TLSTM steps on host (numpy fp32). x_seq: [T,B,I],
    dt_seq: [T,B]. Returns ys [T,B,H]."""
    T = x_seq.shape[0]
    ys = np.empty((T, x_seq.shape[1], Wd.shape[0]), np.float32)
    for t in range(T):
        c_s = np.tanh(c @ Wd.T + bd)
        c_adj = c - c_s + c_s / np.log(E + dt_seq[t][:, None])
        gates = x_seq[t] @ Wih.T + bihh + h @ Whh.T
        i_g, f_g, g_g, o_g = np.split(gates, 4, axis=-1)
        c = _sigmoid(f_g) * c_adj + _sigmoid(i_g) * np.tanh(g_g)
        h = _sigmoid(o_g) * np.tanh(c)
        ys[t] = h
    return ys


def _stream_rho(dt_dir, wA, wB):
    """rho tile [128, L] for a stream packing windows (rows 0-63 = wA)."""
    rho = np.zeros((128, L), np.float32)
    for j, w in enumerate((wA, wB)):
        t0 = w * L_KEEP - WARM
        sl = slice(64 * j, 64 * (j + 1))
        lo = max(0, -t0)           # pad steps at the head (window 0 only)
        hi = min(L, dt_dir.shape[0] - t0)
        if hi > lo:
            r = 1.0 / np.log(E + dt_dir[t0 + lo : t0 + hi])  # [n, B]
            rho[sl, lo:hi] = (r - 1.0).T
    return rho


def kernel(**inputs):
    from concourse.bass_utils import run_bass_kernel_spmd

    x = np.asarray(inputs["x"], np.float32)
    h0 = np.asarray(inputs["h0"], np.float32)
    c0 = np.asarray(inputs["c0"], np.float32)
    dt_sb = np.asarray(inputs["delta_ts"], np.float32).T  # [S, B]

    wsets = []
    for dsuf in ("f", "r"):
        Wih = np.asarray(inputs[f"W_ih_{dsuf}"], np.float32)
        Whh = np.asarray(inputs[f"W_hh_{dsuf}"], np.float32)
        bihh = (
            np.asarray(inputs[f"b_ih_{dsuf}"], np.float32)
            + np.asarray(inputs[f"b_hh_{dsuf}"], np.float32)
        )
        Wd = np.asarray(inputs[f"W_d_{dsuf}"], np.float32)
        bd = np.asarray(inputs[f"b_d_{dsuf}"], np.float32)
        wsets.append((Wih, Whh, bihh, Wd, bd))

    dt_dirs = [dt_sb, dt_sb[::-1]]

    # x pool, feature-major: xT padded to [I, S + 2*WARM] then per-core slices
    xT = np.zeros((I, S + 2 * WARM, B), BF16)
    xT[:, WARM : WARM + S] = x.transpose(2, 0, 1)

    nc = _get_program(L)

    in_maps = []
    meta = []
    for core in range(NCORES):
        j = core
        pool = xT[:, 256 * j : 256 * j + POOL]  # already WARM-shifted by pad
        m = {"xpool": np.ascontiguousarray(pool.reshape(128, POOL * 64))}
        # stream 0: forward windows (2j, 2j+1); stream 1: reverse (15-2j, 14-2j)
        wins = [(2 * j, 2 * j + 1), (15 - 2 * j, 14 - 2 * j)]
        for s, d in enumerate((0, 1)):
            Wih, Whh, bihh, Wd, bd = wsets[d]
            m[f"wih{s}"] = np.ascontiguousarray(Wih[_PERM].T).astype(BF16)
            m[f"whh{s}"] = np.ascontiguousarray(Whh[_PERM].T).astype(BF16)
            m[f"wd{s}"] = np.ascontiguousarray(Wd.T).astype(BF16)
            m[f"bg{s}"] = np.ascontiguousarray(bihh[_PERM][None, :])
            m[f"bd{s}"] = np.ascontiguousarray(bd[None, :]).astype(BF16)
            m[f"rho{s}"] = _stream_rho(dt_dirs[d], *wins[s])
        in_maps.append(m)
        meta.append(wins)

    global _last_in_maps
    _last_in_maps = in_maps
    res = run_bass_kernel_spmd(nc, in_maps, list(range(NCORES)))

    out = np.empty((S, B, 2 * H), np.float32)
    for core in range(NCORES):
        wins = meta[core]
        for s, d in enumerate((0, 1)):
            raw = res.results[core][f"hs{s}"]
            if OUT_I8:
                q = np.asarray(raw, np.float32)
                hsd = np.arctanh(
                    np.clip(q / 127.0, -126.9 / 127.0, 126.9 / 127.0)
                ) / ALPHA
            else:
                hsd = np.asarray(raw, np.float32)
            hs = hsd.reshape(128, L_KEEP, 128)
            for j, w in enumerate(wins[s]):
                ys = hs[64 * j : 64 * (j + 1)].transpose(1, 0, 2)  # [KEEP, B, H]
                p_lo = w * L_KEEP  # kept positions, direction-local
                if d == 0:
                    out[p_lo : p_lo + L_KEEP, :, 0:H] = ys
                else:
                    orig_hi = S - 1 - p_lo
                    orig_lo = S - 1 - (p_lo + L_KEEP)
                    out[orig_hi : None if orig_lo < 0 else orig_lo : -1,
                        :, H : 2 * H] = ys
    # host fixup: first WARM kept steps of window 0, each direction (exact;
    # these are the only outputs that truly depend on h0/c0)
    x_dirs = [x, x[::-1]]
    for d in range(2):
        Wih, Whh, bihh, Wd, bd = wsets[d]
        ys = _host_scan(
            x_dirs[d][0:WARM], dt_dirs[d][0:WARM],
            h0[d].copy(), c0[d].copy(), Wih, Whh, bihh, Wd, bd
        )
        if d == 0:
            out[0:WARM, :, 0:H] = ys
        else:
            out[S - 1 : S - 1 - WARM : -1, :, H : 2 * H] = ys
    return out
